# revision 10
# baseline (speedup 1.0000x reference)
"""Causal self-attention Trainium2 kernel (8 NeuronCores).

Sharding: tensor-parallel over heads x data-parallel over batch.
Core c handles batch b = c // 4 and head group g = c % 4 (4 heads of 16).
Each core computes q/k/v projections for its heads, causal attention, and a
partial output projection (its 256 columns of the 1024-wide contraction);
the host sums the 4 partials per batch.

Layout strategy (all transpose-free on device):
  - q,k are projected directly in transposed layout qkT[e, t] (e on
    partitions) so they feed the scores matmul as lhsT/rhs.
  - scores are computed transposed, sT[k_chunk=128, q_block=512], one
    matmul per (k_chunk, q_block) with K=hd=64.
  - softmax: no max-subtraction (scores ~ N(0,1), exp is safe in fp32);
    exp on ScalarE reading PSUM; causal mask added as -1e9 bias into PSUM
    for diagonal chunks; denominator comes free as an extra ones-column in
    the PV matmul's lhsT.
  - v is projected in natural layout v[t, hd] which is exactly the PV lhsT.
  - PV output yT[hd, q] is normalized via a K=1 broadcast matmul of the
    reciprocal row, then used directly as the proj lhsT.
All matmuls run as float32r (full PE rate at N>=256).
"""

import numpy as np

import concourse.bass as bass
from concourse import bacc
import concourse.mybir as mybir
import concourse.tile as tile
from concourse.bass_utils import run_bass_kernel_spmd

B, T, D, H = 2, 2048, 1024, 16
HD = D // H          # 64
HPC = 4              # heads per core
NCORES = 8
EQK = 2 * HPC * HD   # 512 rows of q+k per core
EV = HPC * HD        # 256 rows of v per core
TB = 512             # t/q block
NTB = T // TB        # 4
TC = 128             # t chunk
NTC = T // TC        # 16
DCH = D // 128       # 8 contraction chunks
F32 = mybir.dt.float32
F32R = mybir.dt.float32r

_cache = {}


def _ensure_ntff_hook():
    """The agent image's antenv lacks axon_hooks; fabricate it so
    run_bass_kernel_spmd(trace=True) can capture NTFF profiles."""
    import sys
    import types
    try:
        import antenv.axon_hooks  # noqa: F401
        return
    except ImportError:
        pass
    try:
        import antenv
        from trn_agent_boot.trn_boot import _ntff_profile_via_ctypes
        hook = {"h": _ntff_profile_via_ctypes("/opt/axon/libaxon_pjrt.so")}
        m = types.ModuleType("antenv.axon_hooks")
        m.get_axon_ntff_profile_hook = lambda: hook["h"]
        m.set_axon_ntff_profile_hook = lambda h: hook.update(h=h)
        sys.modules["antenv.axon_hooks"] = m
        antenv.axon_hooks = m
    except Exception:
        pass


def _build_nc():
    nc = bacc.Bacc("TRN2", target_bir_lowering=False, debug=False,
                  num_devices=NCORES)
    xT = nc.dram_tensor("xT", [D, T], F32R, kind="ExternalInput")
    wqk = nc.dram_tensor("wqk", [D, EQK], F32R, kind="ExternalInput")
    wv = nc.dram_tensor("wv", [D, EV], F32R, kind="ExternalInput")
    wp = nc.dram_tensor("wp", [EV, D], F32R, kind="ExternalInput")
    masks = nc.dram_tensor("masks", [4, 128, TB], F32, kind="ExternalInput")
    onesd = nc.dram_tensor("onesd", [128, HD], F32R, kind="ExternalInput")
    out = nc.dram_tensor("out", [T, D], F32, kind="ExternalOutput")

    with tile.TileContext(nc) as tc:
        with (
            nc.allow_low_precision(reason="fp32r matmul inputs; psum stays fp32"),
            tc.tile_pool(name="persist", bufs=1) as persist,
            tc.tile_pool(name="xin", bufs=2) as xin,
            tc.tile_pool(name="work", bufs=3) as work,
            tc.tile_pool(name="outp", bufs=3) as outp,
            tc.tile_pool(name="ps_big", bufs=3, space="PSUM") as ps_big,
            tc.tile_pool(name="ps_acc", bufs=2, space="PSUM") as ps_acc,
            tc.tile_pool(name="ps_sm", bufs=2, space="PSUM") as ps_sm,
        ):
            # ---- persistent SBUF tensors ----
            wqk_sb = persist.tile([128, DCH, EQK], F32R)   # 16KB/part
            nc.sync.dma_start(wqk_sb[:], wqk.rearrange("(c p) e -> p c e", p=128))
            wv_sb = persist.tile([128, DCH, EV], F32R)     # 8KB/part
            nc.sync.dma_start(wv_sb[:], wv.rearrange("(c p) e -> p c e", p=128))
            wp_sb = persist.tile([128, 2, D], F32R)        # 8KB/part
            nc.sync.dma_start(wp_sb[:], wp.rearrange("(c p) e -> p c e", p=128))
            mask_sb = persist.tile([128, 4, TB], F32)     # 8KB/part
            nc.sync.dma_start(mask_sb[:], masks.rearrange("j p q -> p j q"))
            ones_sb = persist.tile([1, HD], F32R)
            nc.sync.dma_start(ones_sb[:], onesd[0:1, :])

            # qkT[e, t]: 4 chunks of 128 e-rows (q heads 01, q heads 23,
            # k heads 01, k heads 23), each [128, T]
            qkT = [persist.tile([128, T], F32R, tag=f"qkT{i}", name=f"qkT{i}")
                   for i in range(4)]
            # v_sb[t_chunk]: [128, h, 65]; col 64 of each head slot is 1.0
            v_sb = [persist.tile([128, HPC, HD + 1], F32R, tag=f"v{i}",
                                name=f"v{i}")
                    for i in range(NTC)]
            # yT: unnormalized-then-normalized attention output, [hd_all, t]
            yT = [persist.tile([128, T], F32R, tag=f"yT{i}", name=f"yT{i}")
                  for i in range(2)]

            def qT_ap(h):  # [64, T]
                return qkT[h // 2][64 * (h % 2):64 * (h % 2) + 64, :]

            def kT_ap(h):  # [64, T]
                return qkT[2 + h // 2][64 * (h % 2):64 * (h % 2) + 64, :]

            # ================= QKV projection =================
            for b in range(NTB):
                x_t = xin.tile([128, DCH, TB], F32R, tag="x")
                nc.sync.dma_start(
                    x_t[:], xT[:, b * TB:(b + 1) * TB]
                    .rearrange("(c p) t -> p c t", p=128))
                # q,k in transposed layout: psum[e_chunk 128, t 512]
                for ec in range(4):
                    ps = ps_big.tile([128, TB], F32, tag="mm", name="ps_qk")
                    for dc in range(DCH):
                        nc.tensor.matmul(
                            ps[:],
                            (wqk_sb[:, dc, 128 * ec:128 * (ec + 1)]),
                            (x_t[:, dc, :]),
                            start=(dc == 0), stop=(dc == DCH - 1))
                    nc.scalar.copy(qkT[ec][:, b * TB:(b + 1) * TB], ps[:])
                # v in natural layout: psum[t_chunk 128, hd 256]
                for t2 in range(4):
                    tc_i = 4 * b + t2
                    ps = ps_big.tile([128, TB], F32, tag="mm", name="ps_v")
                    for dc in range(DCH):
                        nc.tensor.matmul(
                            ps[:, 0:EV],
                            (x_t[:, dc, 128 * t2:128 * (t2 + 1)]),
                            (wv_sb[:, dc, :]),
                            start=(dc == 0), stop=(dc == DCH - 1))
                    nc.vector.tensor_copy(
                        v_sb[tc_i][:, :, 0:HD],
                        ps[:, 0:EV].rearrange("p (h f) -> p h f", h=HPC))
                    nc.sync.dma_start(v_sb[tc_i][:, :, HD], onesd[:, 0:HPC])

            # ================= attention =================
            for h in range(HPC):
                for b in range(NTB):
                    nk = 4 * b + 4
                    ps_pv = ps_acc.tile([HD + 1, TB], F32, tag="pv")
                    for kc in range(nk):
                        ps_s = ps_big.tile([128, TB], F32, tag="mm", name="ps_s")
                        nc.tensor.matmul(
                            ps_s[:],
                            (kT_ap(h)[:, 128 * kc:128 * (kc + 1)]),
                            (qT_ap(h)[:, b * TB:(b + 1) * TB]),
                            start=True, stop=True)
                        if kc >= 4 * b:
                            nc.vector.tensor_add(
                                ps_s[:], ps_s[:], mask_sb[:, kc - 4 * b, :])
                        probs = work.tile([128, TB], F32R, tag="probs")
                        nc.scalar.activation(
                            probs[:], ps_s[:],
                            mybir.ActivationFunctionType.Exp,
                            scale=1.0 / np.sqrt(HD))
                        nc.tensor.matmul(
                            ps_pv[:],
                            (v_sb[kc][:, h, :]),
                            (probs[:]),
                            start=(kc == 0), stop=(kc == nk - 1))
                    recip = work.tile([1, TB], F32R, tag="recip")
                    nc.vector.reciprocal(recip[:], ps_pv[HD:HD + 1, :])
                    ps_bc = ps_sm.tile([HD, TB], F32, tag="bc")
                    nc.tensor.matmul(
                        ps_bc[:], (ones_sb[:]), (recip[:]),
                        start=True, stop=True)
                    bc_sb = work.tile([HD, TB], F32, tag="bc_sb")
                    nc.scalar.copy(bc_sb[:], ps_bc[:])
                    nc.vector.tensor_mul(
                        yT[h // 2][64 * (h % 2):64 * (h % 2) + 64,
                                   b * TB:(b + 1) * TB],
                        ps_pv[0:HD, :], bc_sb[:])

            # ================= output projection =================
            for tc_i in range(NTC):
                for e in range(2):
                    ps = ps_big.tile([128, TB], F32, tag="mm", name="ps_proj")
                    for c in range(2):
                        nc.tensor.matmul(
                            ps[:],
                            (yT[c][:, 128 * tc_i:128 * (tc_i + 1)]),
                            (wp_sb[:, c, 512 * e:512 * (e + 1)]),
                            start=(c == 0), stop=(c == 1))
                    o_sb = outp.tile([128, TB], F32, tag="o")
                    nc.vector.tensor_copy(o_sb[:], ps[:])
                    nc.sync.dma_start(
                        out[128 * tc_i:128 * (tc_i + 1),
                            512 * e:512 * (e + 1)], o_sb[:])
    nc.compile()
    return nc


def _masks_np():
    m = np.zeros((4, 128, TB), dtype=np.float32)
    kr = np.arange(128)[:, None]
    qc = np.arange(TB)[None, :]
    for j in range(4):
        m[j] = np.where(kr <= qc - 128 * j, 0.0, -1e9).astype(np.float32)
    return m


def _prep_in_maps(x, w_qkv, w_proj):
    masks = _masks_np()
    in_maps = []
    for c in range(NCORES):
        b, g = c // 4, c % 4
        heads = slice(g * HPC * HD, (g + 1) * HPC * HD)      # 256 rows
        wq = w_qkv[0 * D:1 * D][heads]                        # [256, 1024]
        wk = w_qkv[1 * D:2 * D][heads]
        wv = w_qkv[2 * D:3 * D][heads]
        in_maps.append({
            "xT": np.ascontiguousarray(x[b].T),               # [1024, 2048]
            "wqk": np.ascontiguousarray(
                np.concatenate([wq, wk], axis=0).T),          # [1024, 512]
            "wv": np.ascontiguousarray(wv.T),                 # [1024, 256]
            "wp": np.ascontiguousarray(w_proj[:, heads].T),   # [256, 1024]
            "masks": masks,
            "onesd": np.ones((128, HD), dtype=np.float32),
        })
    return in_maps


def kernel(x, w_qkv, w_proj, _trace=False):
    x = np.asarray(x, dtype=np.float32)
    w_qkv = np.asarray(w_qkv, dtype=np.float32)
    w_proj = np.asarray(w_proj, dtype=np.float32)
    if _trace:
        _ensure_ntff_hook()
    if "nc" not in _cache:
        _cache["nc"] = _build_nc()
    nc = _cache["nc"]
    in_maps = _prep_in_maps(x, w_qkv, w_proj)
    res = run_bass_kernel_spmd(nc, in_maps, list(range(NCORES)),
                               trace=_trace)
    out = np.zeros((B, T, D), dtype=np.float32)
    for c in range(NCORES):
        out[c // 4] += res.results[c]["out"]
    if _trace:
        _cache["last_result"] = res
    return out
